# revision 50
# baseline (speedup 1.0000x reference)
"""Multi-head causal self-attention on 8 TRN2 NeuronCores.

Problem (hardcoded): x[2,2048,1024] f32, Q/K/V/O [1024,1024] f32, 16 heads,
Dh=64, causal softmax, out = attn(x) @ O.T  -> [2,2048,1024] f32.

Sharding: core c handles batch b=c//4 and head group g=c%4 (4 heads each).
Each core computes a partial output (its heads' contribution through the O
projection); the host gather sums the 4 partials per batch (the all-reduce
of the hint, performed at unshard time).

Device algorithm per core (heads h=0..3):
  Phase 1 (cadenced with the x DMA stream, 8 chunks of 256 columns):
      packed q/k projections with fp32r matmuls; psum chunks are copied
      into packed [128,S] staging tiles read by the A-pass via
      tile_position. A-pass blocks for heads 0/1 are emitted per chunk so
      PE never waits on the stream. One DMA per head then splits pack rows
      into per-head 65-row tiles for the T-pass: row 64 of kt2[h] = ones,
      row 64 of qt2[h] = -rowmax (negrow, DMA'd from the A-pass result).
  Phase 2 per head (T-passes explicitly interleaved with A/av/ph3 filler
  work, since each engine executes its instructions in emission order):
      T-pass: scores_T[k,q] fp32r with K=65: row 64 contributes
          ones * (-max_q), i.e. the max subtraction fused into the matmul;
          diag causal mask add via PE; ACT exp -> PT bf16 (k-major).
      A-pass (heads 2/3, during T(0)/T(1)): scores fp32r K=64 + PE diag
          mask; row max via DVE tensor_tensor_reduce (pairs of chunks per
          instruction, negated via scale=-1/min).
      av: out[q,d]+denominator via PT.T @ [v|1], 1/l via DVE reciprocal,
          normalize on Pool.
  Phase 3 (interleaved into the T(3) window as PT columns become ready):
      ho[s,hd] -PE transpose-> hoT, out_partial = hoT.T @ O_cols.T (bf16).
"""
import numpy as np

import concourse.bass as bass
import concourse.tile as tile
from concourse import bacc, mybir
from concourse.bass_utils import run_bass_kernel_spmd
from concourse.masks import make_identity

F32 = mybir.dt.float32
F32R = mybir.dt.float32r
BF16 = mybir.dt.bfloat16

B, S, D, H = 2, 2048, 1024, 16
DH = 64          # head dim
HPC = 4          # heads per core
NB = S // 128    # 16 q/k blocks
FT = D // 128    # 8 f-tiles
NC_CH = 8        # x DMA chunks (256 cols each)
NEG = -3.0e38

# PT column offsets: head-local P^T storage, block j spans q-cols [j*128, S)
PT_OFF = [0] * (NB + 1)
for _j in range(NB):
    PT_OFF[_j + 1] = PT_OFF[_j] + (S - _j * 128)
PT_COLS = PT_OFF[NB]  # 17408


def _chunks(w):
    """Split width w into matmul chunks, avoiding <256 chunks when possible
    (fp32r runs 4x slower below 256 output columns)."""
    if w <= 512:
        return [w]
    out = [512] * (w // 512)
    rem = w % 512
    if rem == 128:
        out[-1] = 384
        out.append(256)
    elif rem:
        out.append(rem)
    return out


def _a_chunks(w):
    return _chunks(w)


def _interleave(primary, fillers):
    """Emit primary thunks in order, spreading filler thunks between them
    (engines execute their instruction streams in emission order, so this
    is what makes PE fill T-pass exp-wait bubbles with other matmuls)."""
    n, m = len(primary), len(fillers)
    fi = 0
    for k, thunk in enumerate(primary):
        thunk()
        want = (k + 1) * m // n
        while fi < want:
            fillers[fi]()
            fi += 1
    while fi < m:
        fillers[fi]()
        fi += 1


def build_nc():
    nc = bacc.Bacc(None, target_bir_lowering=False, debug=False)

    xt_d = nc.dram_tensor("xt", [D, S], F32, kind="ExternalInput")
    qt_d = nc.dram_tensor("qt", [D, 256], F32, kind="ExternalInput")
    kt_d = nc.dram_tensor("kt", [D, 256], F32, kind="ExternalInput")
    vt_d = nc.dram_tensor("vt", [D, 256], F32, kind="ExternalInput")
    ot_d = nc.dram_tensor("ot", [256, D], F32, kind="ExternalInput")
    tri_d = nc.dram_tensor("tri", [128, 128], F32, kind="ExternalInput")
    rsh_d = nc.dram_tensor("rsh", [128, 128], F32, kind="ExternalInput")
    out_d = nc.dram_tensor("out", [S, D], F32, kind="ExternalOutput")

    with tile.TileContext(nc) as tc:
        with (
            tc.tile_pool(name="singles", bufs=1) as singles,
            tc.tile_pool(name="mid", bufs=1) as mid,
        ):
            # whole-kernel constants / tensors
            v_sb = [singles.tile([128, HPC, 65], BF16, name=f"v{j}")
                    for j in range(NB)]
            ho_sb = [singles.tile([128, 256], BF16, name=f"ho{i}")
                     for i in range(NB)]
            tri_sb = singles.tile([128, 128], BF16)
            rsh_sb = singles.tile([128, 128], BF16)
            identf = singles.tile([128, 128], F32)
            identb = singles.tile([128, 128], BF16)
            ones_row = singles.tile([128, 16], F32)

            nc.gpsimd.dma_start(tri_sb[:], tri_d[:])
            nc.gpsimd.dma_start(rsh_sb[:], rsh_d[:])
            for j in range(NB):
                nc.gpsimd.memset(v_sb[j][:, :, 64:65], 1.0)
            nc.gpsimd.memset(ones_row[:], 1.0)
            make_identity(nc, identf[:])
            make_identity(nc, identb[:])

            # per-head cross-phase tiles: rows 0:64 = head data (f32r),
            # row 64 of kt2 = ones, row 64 of qt2 = -rowmax (negrow)
            qt2 = [mid.tile([65, S], F32R, name=f"qt2_{h}") for h in range(HPC)]
            kt2 = [mid.tile([65, S], F32R, name=f"kt2_{h}") for h in range(HPC)]

            # ---------------- pools ----------------
            ph2_cm = tc.tile_pool(name="ph2", bufs=2)
            psA_cm = tc.tile_pool(name="psA", bufs=2, space="PSUM")
            ph2, psA = ph2_cm.__enter__(), psA_cm.__enter__()
            pack_cm = tc.tile_pool(name="packs", bufs=1)
            pack_pool = pack_cm.__enter__()
            ph1_cm = tc.tile_pool(name="ph1", bufs=1)
            pp_cm = tc.tile_pool(name="pp", bufs=4, space="PSUM")
            ph1, pp = ph1_cm.__enter__(), pp_cm.__enter__()

            # x stored as one [128, 8, 2048] tile so later chunks can each
            # be a single big DMA (HWDGE issue rate, ~625ns/DMA, limits many
            # small DMAs).
            xt_big = ph1.tile([128, FT, S], F32R, name="xt_big")
            qtw = ph1.tile([128, FT, 256], F32R)
            ktw = ph1.tile([128, FT, 256], F32R)
            vtw = ph1.tile([128, FT, 256], F32R)

            # Input stream, all on the SP queue (keeps ACT/DVE/Pool free):
            # qt/kt halves interleaved with x chunk 0 per-t tiles (locksteps
            # the first proj chains), then x chunks 1..7 as one DMA each,
            # then vtw (v-proj runs last, overlapping T(0)), then ones rows.
            def dma_w_half(w_sb, w_d, lo):
                nc.sync.dma_start(
                    w_sb[:, lo:lo + 4, :],
                    w_d[lo * 128:(lo + 4) * 128, :]
                    .rearrange("(t p) m -> p t m", p=128).bitcast(F32R))

            def dma_w_q(w_sb, w_d, lo):
                nc.sync.dma_start(
                    w_sb[:, lo:lo + 2, :],
                    w_d[lo * 128:(lo + 2) * 128, :]
                    .rearrange("(t p) m -> p t m", p=128).bitcast(F32R))

            for lo in range(0, FT, 2):
                dma_w_q(qtw, qt_d, lo)
                dma_w_q(ktw, kt_d, lo)
                for t in (lo, lo + 1):
                    nc.sync.dma_start(
                        xt_big[:, t, 0:256],
                        xt_d[t * 128:(t + 1) * 128, 0:256].bitcast(F32R))
            for c in range(1, NC_CH):
                cs = slice(c * 256, (c + 1) * 256)
                nc.sync.dma_start(
                    xt_big[:, :, cs],
                    xt_d[:, cs].rearrange("(t p) m -> p t m", p=128)
                    .bitcast(F32R))
            dma_w_half(vtw, vt_d, 0)
            dma_w_half(vtw, vt_d, 4)
            for h in range(HPC):
                nc.sync.dma_start(kt2[h][64:65, :], ones_row[:].bitcast(F32R))

            pts = {}
            negmaxs = {}
            packs_all = {}
            for p in range(2):
                for ty in (0, 1):
                    packs_all[(ty, p)] = pack_pool.tile(
                        [128, S], F32R, tag="pack", name=f"pack{ty}_{p}",
                        bufs=4)

            def emit_proj_chunk(c):
                # all 4 (type, pair) q/k chains for S-cols [c*256,(c+1)*256).
                # pair p covers heads 2p (psum rows 0:64) / 2p+1 (64:128).
                cs = slice(c * 256, (c + 1) * 256)
                chains = []
                for p in (0, 1):
                    for w_sb, ty in ((qtw, 0), (ktw, 1)):
                        ps = pp.tile([128, 256], F32, tag="ps", name="ps")
                        chains.append((ps, w_sb, ty, p))
                for t in range(FT):
                    for ps, w_sb, ty, p in chains:
                        nc.tensor.matmul(
                            ps[:],
                            w_sb[:, t, p * 128:(p + 1) * 128],
                            xt_big[:, t, cs],
                            start=(t == 0), stop=(t == FT - 1),
                        )
                for idx, (ps, w_sb, ty, p) in enumerate(chains):
                    nc.scalar.copy(packs_all[(ty, p)][:, cs],
                                   ps[:].bitcast(F32R))

            def emit_vproj_block(sb_i):
                ps = pp.tile([128, 256], F32, tag="ps", name="vps")
                for t in range(FT):
                    nc.tensor.matmul(
                        ps[:],
                        xt_big[:, t, sb_i * 128:(sb_i + 1) * 128],
                        vtw[:, t, :],
                        start=(t == 0), stop=(t == FT - 1),
                    )
                nc.scalar.copy(
                    v_sb[sb_i][:, :, 0:64],
                    ps[:].rearrange("p (h d) -> p h d", d=64),
                )

            def emit_split(h):
                # per-head 65-row tiles: rows 0:64 from the packed staging
                # tiles (row 64 holds ones / negrow). Split per 512-col
                # piece so each DMA depends on exactly the pack copies that
                # produced its columns.
                p, po = h // 2, 64 * (h % 2)
                for ty, dstl in ((0, qt2), (1, kt2)):
                    pk = packs_all[(ty, p)]
                    nc.sync.dma_start(dstl[h][0:64, :], pk[po:po + 64, :])

            def emit_A_block(h, i):
                if h < 2:
                    pk_q = packs_all[(0, h // 2)]
                    pk_k = packs_all[(1, h // 2)]
                    po = 64 * (h % 2)
                else:
                    pk_q, pk_k, po = qt2[h], kt2[h], 0
                negmax = negmaxs[h]
                w = (i + 1) * 128
                cks = _a_chunks(w)
                nch = len(cks)
                mp = ph2.tile([128, 4], F32, tag="maxpart",
                              name=f"mp{h}", bufs=6)
                tiles = []
                co = 0
                for c, cw in enumerate(cks):
                    sA = psA.tile([128, 512], F32, tag="sA", name=f"sA{h}",
                                  bufs=3)
                    tiles.append((sA, cw))
                    nc.tensor.matmul(
                        sA[:, 0:cw],
                        pk_q[po:po + 64, i * 128:(i + 1) * 128],
                        pk_k[po:po + 64, co:co + cw],
                        start=True, stop=True,
                        tile_position=(po, 0),
                    )
                    if c == nch - 1:  # diag: += -BIG*[k>q] via PE
                        dlo = cw - 128
                        nc.tensor.matmul(
                            sA[:, dlo:dlo + 128],
                            rsh_sb[:], tri_sb[:],
                            start=False, stop=True,
                            skip_group_check=True)
                    co += cw
                if nch == 1:
                    a, acw = tiles[0]
                    nc.vector.reduce_max(
                        negmax[:, i:i + 1], a[:, 0:acw],
                        axis=mybir.AxisListType.X, negate=True)
                else:
                    for c, (a, acw) in enumerate(tiles):
                        nc.vector.reduce_max(
                            mp[:, c:c + 1], a[:, 0:acw],
                            axis=mybir.AxisListType.X)
                    nc.vector.reduce_max(
                        negmax[:, i:i + 1], mp[:, 0:nch],
                        axis=mybir.AxisListType.X, negate=True)

            def emit_negmax_finish(h):
                # negmax -> negrow [1, S] stored as row 64 of qt2[h]
                negmax = negmaxs[h]
                pst = psA.tile([16, 128], F32, tag="sA", bufs=3)
                nc.tensor.transpose(pst[:], negmax[:], identf[:])
                stage = ph2.tile([16, 128], F32, tag="stage", bufs=2)
                nc.vector.tensor_copy(stage[:], pst[:])
                nc.sync.dma_start(qt2[h][64:65, :], stage[:].bitcast(F32R))

            def A_thunks(h):
                negmaxs[h] = ph2.tile([128, NB], F32, tag="negmax",
                                      name=f"negmax{h}", bufs=4)
                out = [(lambda hh=h, ii=i: emit_A_block(hh, ii))
                       for i in range(NB)]
                out.append(lambda hh=h: emit_negmax_finish(hh))
                return out

            def T_thunks(h):
                pt = pt_pool.tile([128, PT_COLS], BF16, tag="pt",
                                  name=f"pt{h}")
                pts[h] = pt

                def tile_thunk(j, t0, tw):
                    def f():
                        sT = psT.tile([128, 1024], F32, tag="sT",
                                      name=f"sT{h}")
                        coff = 0
                        # bank-aligned 512 splits: a matmul's psum output
                        # must not cross a 2KB bank boundary
                        tcks = [512] * (tw // 512)
                        if tw % 512:
                            tcks.append(tw % 512)
                        for cw in tcks:
                            diag = (t0 == j * 128 and coff == 0)
                            # K=65: row 64 = ones (kt2) x negrow (qt2)
                            # -> scores - rowmax, fused
                            nc.tensor.matmul(
                                sT[:, coff:coff + cw],
                                kt2[h][0:65, j * 128:(j + 1) * 128],
                                qt2[h][0:65, t0 + coff:t0 + coff + cw],
                                start=True, stop=not diag,
                            )
                            if diag:  # += -BIG*[q<k] via PE
                                nc.tensor.matmul(
                                    sT[:, 0:128],
                                    tri_sb[:], rsh_sb[:],
                                    start=False, stop=True,
                                    skip_group_check=True)
                            coff += cw
                        nc.scalar.activation(
                            pt[:, PT_OFF[j] + t0 - j * 128:
                               PT_OFF[j] + t0 - j * 128 + tw],
                            sT[:, 0:tw],
                            mybir.ActivationFunctionType.Exp)
                    return f

                out = []   # list of (j_completed_after, thunk)
                for j in range(NB):
                    t0 = j * 128
                    while t0 < S:
                        tw = min(1024, S - t0)
                        last = (t0 + tw == S)
                        out.append((j if last else j - 1,
                                    tile_thunk(j, t0, tw)))
                        t0 += tw
                return out

            def emit_av_block(h, i, with_ph3):
                pt = pts[h]
                av = av2[:, i % 3, :]
                for j in range(i + 1):
                    nc.tensor.matmul(
                        av[:],
                        pt[:, PT_OFF[j] + (i - j) * 128:
                           PT_OFF[j] + (i - j) * 128 + 128],
                        v_sb[j][:, h, :],
                        start=(j == 0), stop=(j == i),
                    )
                recip = ph2.tile([128, 1], F32, tag="recip", bufs=6)
                nc.vector.reciprocal(recip[:], av[:, 64:65])
                nc.vector.tensor_scalar_mul(
                    ho_sb[i][:, h * 64:(h + 1) * 64],
                    av[:, 0:64], recip[:])
                if with_ph3:
                    emit_ph3(i)

            def av_thunks(h, with_ph3=False):
                return [(lambda hh=h, ii=i, w3=with_ph3:
                         emit_av_block(hh, ii, w3)) for i in range(NB)]

            def emit_ph3(i):
                hot = ph3.tile([128, 256], BF16, tag="hot", name="hot",
                               bufs=2)
                for t in range(2):
                    ptile = psA.tile([128, 128], BF16, tag="sA",
                                     name="ptile", bufs=3)
                    nc.tensor.transpose(
                        ptile[:], ho_sb[i][:, t * 128:(t + 1) * 128],
                        identb[:])
                    nc.vector.tensor_copy(hot[:, t * 128:(t + 1) * 128],
                                          ptile[:])
                ostage = ph3.tile([128, D], F32, tag="ostage", name="ostage",
                                  bufs=2)
                for nchunk in range(2):
                    pot = psA.tile([128, 512], F32, tag="sA", name="pot",
                                   bufs=3)
                    for t in range(2):
                        nc.tensor.matmul(
                            pot[:],
                            hot[:, t * 128:(t + 1) * 128],
                            ot_sb[:, t, nchunk * 512:(nchunk + 1) * 512],
                            start=(t == 0), stop=(t == 1),
                        )
                    ocs = slice(nchunk * 512, (nchunk + 1) * 512)
                    if nchunk == 0:
                        nc.scalar.copy(ostage[:, ocs], pot[:])
                    else:
                        nc.vector.tensor_copy(ostage[:, ocs], pot[:])
                    nc.sync.dma_start(out_d[i * 128:(i + 1) * 128, ocs],
                                      ostage[:, ocs])

            # ---------------- phase 1: cadenced with the x stream --------
            # per chunk c: ready v-proj blocks first (front-loads deferrable
            # work), then q/k chains (gated by chunk c's DMA), then the
            # A(0)/A(1) blocks whose k-range chunk c completes. This keeps
            # the post-stream PE backlog minimal so T(0)'s exp can start as
            # soon as negrow(0) lands.
            a01 = {0: A_thunks(0), 1: A_thunks(1)}
            for c in range(NC_CH):
                emit_proj_chunk(c)
                if c < NC_CH - 1:
                    for h in (0, 1):
                        a01[h][2 * c]()
                        a01[h][2 * c + 1]()
            for h in (0, 1):
                a01[h][2 * (NC_CH - 1)]()
                a01[h][2 * (NC_CH - 1) + 1]()
            # splits first: they depend only on the packs (ready early) and
            # must not queue behind the negrow DMAs on the SP queue
            for h in range(HPC):
                emit_split(h)
            for h in (0, 1):
                a01[h][NB]()               # negmax finish

            # v-proj last: it gates only av(0), so it overlaps the
            # negrow/split latency and the start of T(0)'s exp stream.
            for sb_i in range(NB):
                emit_vproj_block(sb_i)
            pp_cm.__exit__(None, None, None)    # frees 4 PSUM banks
            ph1_cm.__exit__(None, None, None)   # frees x/weights SBUF
            pack_cm.__exit__(None, None, None)  # frees 32KB of staging

            pt_cm = tc.tile_pool(name="pt_pool", bufs=2)
            ph3_cm = tc.tile_pool(name="ph3", bufs=4)
            psT_cm = tc.tile_pool(name="psT", bufs=2, space="PSUM")
            psV_cm = tc.tile_pool(name="psV", bufs=1, space="PSUM")
            pt_pool, ph3 = pt_cm.__enter__(), ph3_cm.__enter__()
            psT, psV = psT_cm.__enter__(), psV_cm.__enter__()
            av2 = psV.tile([128, 3, 65], F32, tag="av", name="av2", bufs=1)
            ot_sb = ph3.tile([128, 2, D], BF16, tag="ot_sb", bufs=1)
            nc.gpsimd.dma_start(ot_sb[:],
                                ot_d[:].rearrange("(t p) n -> p t n", p=128))

            # ---------------- phase 2: T windows with fillers ------------
            # T(0) window: A(2) blocks fill the exp-paced bubbles
            _interleave([t for _, t in T_thunks(0)], A_thunks(2))
            # T(1) window: av(0) + A(3)
            _interleave([t for _, t in T_thunks(1)],
                        av_thunks(0) + A_thunks(3))
            # T(2) window: av(1)
            _interleave([t for _, t in T_thunks(2)], av_thunks(1))
            # T(3) window: av(2), plus av(3)+ph3 blocks as soon as the PT
            # columns they need are exp'd (j-coverage pacing)
            t3 = T_thunks(3)
            av2_t = av_thunks(2)
            av3_t = av_thunks(3)
            fi2 = 0
            nxt3 = 0
            for k, (jdone, thunk) in enumerate(t3):
                thunk()
                want = (k + 1) * len(av2_t) // len(t3)
                while fi2 < want:
                    av2_t[fi2]()
                    fi2 += 1
                while nxt3 <= jdone:
                    av3_t[nxt3]()
                    if nxt3 >= 1:
                        emit_ph3(nxt3 - 1)
                    nxt3 += 1
            while fi2 < len(av2_t):
                av2_t[fi2]()
                fi2 += 1
            while nxt3 < NB:
                av3_t[nxt3]()
                if nxt3 >= 1:
                    emit_ph3(nxt3 - 1)
                nxt3 += 1
            emit_ph3(NB - 1)

            for cm in (psV_cm, psT_cm, ph3_cm, pt_cm, psA_cm, ph2_cm):
                cm.__exit__(None, None, None)

    nc.compile()
    return nc


_NC_CACHE = None


def _get_nc():
    global _NC_CACHE
    if _NC_CACHE is None:
        _NC_CACHE = build_nc()
    return _NC_CACHE


def kernel(x, Q, K, V, O, num_heads=16, _want_results=False, **run_kwargs):
    x = np.asarray(x, dtype=np.float32)
    Q = np.asarray(Q, dtype=np.float32)
    K = np.asarray(K, dtype=np.float32)
    V = np.asarray(V, dtype=np.float32)
    O = np.asarray(O, dtype=np.float32)
    assert x.shape == (B, S, D) and int(num_heads) == H

    idx = np.arange(128)
    # tri[c,k] = [c<=k]; rsh[c,q] = -BIG*[c==q+1]
    # A-side: (rsh.T@tri)[q,k] = -BIG*[k>q]; T-side: (tri.T@rsh)[k,q] = -BIG*[q<k]
    tri = (idx[:, None] <= idx[None, :]).astype(np.float32)
    rsh = np.zeros((128, 128), dtype=np.float32)
    rsh[idx[1:], idx[:-1]] = NEG

    in_maps = []
    for c in range(8):
        b, g = c // 4, c % 4
        rows = slice(g * 256, (g + 1) * 256)
        in_maps.append(dict(
            xt=np.ascontiguousarray(x[b].T),
            qt=np.ascontiguousarray((Q[rows, :] / 8.0).T),
            kt=np.ascontiguousarray(K[rows, :].T),
            vt=np.ascontiguousarray(V[rows, :].T),
            ot=np.ascontiguousarray(O[:, rows].T),
            tri=tri,
            rsh=rsh,
        ))

    nc = _get_nc()
    res = run_bass_kernel_spmd(nc, in_maps, core_ids=list(range(8)), **run_kwargs)

    out = np.zeros((B, S, D), dtype=np.float32)
    for c in range(8):
        out[c // 4] += res.results[c]["out"]
    if _want_results:
        return out, res
    return out


# revision 53
# speedup vs baseline: 1.0010x; 1.0010x over previous
"""Multi-head causal self-attention on 8 TRN2 NeuronCores.

Problem (hardcoded): x[2,2048,1024] f32, Q/K/V/O [1024,1024] f32, 16 heads,
Dh=64, causal softmax, out = attn(x) @ O.T  -> [2,2048,1024] f32.

Sharding: core c handles batch b=c//4 and head group g=c%4 (4 heads each).
Each core computes a partial output (its heads' contribution through the O
projection); the host gather sums the 4 partials per batch (the all-reduce
of the hint, performed at unshard time).

Device algorithm per core (heads h=0..3):
  Phase 1 (cadenced with the x DMA stream, 8 chunks of 256 columns):
      packed q/k projections with fp32r matmuls; psum chunks are copied
      into packed [128,S] staging tiles read by the A-pass via
      tile_position. A-pass blocks for heads 0/1 are emitted per chunk so
      PE never waits on the stream. One DMA per head then splits pack rows
      into per-head 65-row tiles for the T-pass: row 64 of kt2[h] = ones,
      row 64 of qt2[h] = -rowmax (negrow, DMA'd from the A-pass result).
  Phase 2 per head (T-passes explicitly interleaved with A/av/ph3 filler
  work, since each engine executes its instructions in emission order):
      T-pass: scores_T[k,q] fp32r with K=65: row 64 contributes
          ones * (-max_q), i.e. the max subtraction fused into the matmul;
          diag causal mask add via PE; ACT exp -> PT bf16 (k-major).
      A-pass (heads 2/3, during T(0)/T(1)): scores fp32r K=64 + PE diag
          mask; row max via DVE tensor_tensor_reduce (pairs of chunks per
          instruction, negated via scale=-1/min).
      av: out[q,d]+denominator via PT.T @ [v|1], 1/l via DVE reciprocal,
          normalize on Pool.
  Phase 3 (interleaved into the T(3) window as PT columns become ready):
      ho[s,hd] -PE transpose-> hoT, out_partial = hoT.T @ O_cols.T (bf16).
"""
import numpy as np

import concourse.bass as bass
import concourse.tile as tile
from concourse import bacc, mybir
from concourse.bass_utils import run_bass_kernel_spmd
from concourse.masks import make_identity

F32 = mybir.dt.float32
F32R = mybir.dt.float32r
BF16 = mybir.dt.bfloat16

B, S, D, H = 2, 2048, 1024, 16
DH = 64          # head dim
HPC = 4          # heads per core
NB = S // 128    # 16 q/k blocks
FT = D // 128    # 8 f-tiles
NC_CH = 8        # x DMA chunks (256 cols each)
NEG = -3.0e38

# PT column offsets: head-local P^T storage, block j spans q-cols [j*128, S)
PT_OFF = [0] * (NB + 1)
for _j in range(NB):
    PT_OFF[_j + 1] = PT_OFF[_j] + (S - _j * 128)
PT_COLS = PT_OFF[NB]  # 17408


def _chunks(w):
    """Split width w into matmul chunks, avoiding <256 chunks when possible
    (fp32r runs 4x slower below 256 output columns)."""
    if w <= 512:
        return [w]
    out = [512] * (w // 512)
    rem = w % 512
    if rem == 128:
        out[-1] = 384
        out.append(256)
    elif rem:
        out.append(rem)
    return out


def _a_chunks(w):
    return _chunks(w)


def _interleave(primary, fillers):
    """Emit primary thunks in order, spreading filler thunks between them
    (engines execute their instruction streams in emission order, so this
    is what makes PE fill T-pass exp-wait bubbles with other matmuls)."""
    n, m = len(primary), len(fillers)
    fi = 0
    for k, thunk in enumerate(primary):
        thunk()
        want = (k + 1) * m // n
        while fi < want:
            fillers[fi]()
            fi += 1
    while fi < m:
        fillers[fi]()
        fi += 1


def build_nc():
    nc = bacc.Bacc(None, target_bir_lowering=False, debug=False)

    xt_d = nc.dram_tensor("xt", [D, S], F32, kind="ExternalInput")
    qt_d = nc.dram_tensor("qt", [D, 256], F32, kind="ExternalInput")
    kt_d = nc.dram_tensor("kt", [D, 256], F32, kind="ExternalInput")
    vt_d = nc.dram_tensor("vt", [D, 256], F32, kind="ExternalInput")
    ot_d = nc.dram_tensor("ot", [256, D], F32, kind="ExternalInput")
    tri_d = nc.dram_tensor("tri", [128, 128], F32, kind="ExternalInput")
    rsh_d = nc.dram_tensor("rsh", [128, 128], F32, kind="ExternalInput")
    out_d = nc.dram_tensor("out", [S, D], F32, kind="ExternalOutput")

    with tile.TileContext(nc) as tc:
        with (
            tc.tile_pool(name="singles", bufs=1) as singles,
            tc.tile_pool(name="mid", bufs=1) as mid,
        ):
            # whole-kernel constants / tensors
            v_sb = [singles.tile([128, HPC, 65], BF16, name=f"v{j}")
                    for j in range(NB)]
            ho_sb = [singles.tile([128, 256], BF16, name=f"ho{i}")
                     for i in range(NB)]
            tri_sb = singles.tile([128, 128], BF16)
            rsh_sb = singles.tile([128, 128], BF16)
            identf = singles.tile([128, 128], F32)
            identb = singles.tile([128, 128], BF16)
            ones_row = singles.tile([128, 16], F32)

            nc.gpsimd.dma_start(tri_sb[:], tri_d[:])
            nc.gpsimd.dma_start(rsh_sb[:], rsh_d[:])
            for j in range(NB):
                nc.gpsimd.memset(v_sb[j][:, :, 64:65], 1.0)
            nc.gpsimd.memset(ones_row[:], 1.0)
            make_identity(nc, identf[:])
            make_identity(nc, identb[:])

            # per-head cross-phase tiles: rows 0:64 = head data (f32r),
            # row 64 of kt2 = ones, row 64 of qt2 = -rowmax (negrow)
            qt2 = [mid.tile([65, S], F32R, name=f"qt2_{h}") for h in range(HPC)]
            kt2 = [mid.tile([65, S], F32R, name=f"kt2_{h}") for h in range(HPC)]

            # ---------------- pools ----------------
            ph2_cm = tc.tile_pool(name="ph2", bufs=2)
            psA_cm = tc.tile_pool(name="psA", bufs=2, space="PSUM")
            ph2, psA = ph2_cm.__enter__(), psA_cm.__enter__()
            pack_cm = tc.tile_pool(name="packs", bufs=1)
            pack_pool = pack_cm.__enter__()
            ph1_cm = tc.tile_pool(name="ph1", bufs=1)
            pp_cm = tc.tile_pool(name="pp", bufs=4, space="PSUM")
            ph1, pp = ph1_cm.__enter__(), pp_cm.__enter__()

            # x stored as one [128, 8, 2048] tile so later chunks can each
            # be a single big DMA (HWDGE issue rate, ~625ns/DMA, limits many
            # small DMAs).
            xt_big = ph1.tile([128, FT, S], F32R, name="xt_big")
            qtw = ph1.tile([128, FT, 256], F32R)
            ktw = ph1.tile([128, FT, 256], F32R)
            vtw = ph1.tile([128, FT, 256], F32R)

            # Input stream, all on the SP queue (keeps ACT/DVE/Pool free):
            # qt/kt halves interleaved with x chunk 0 per-t tiles (locksteps
            # the first proj chains), then x chunks 1..7 as one DMA each,
            # then vtw (v-proj runs last, overlapping T(0)), then ones rows.
            def dma_w_half(w_sb, w_d, lo):
                nc.sync.dma_start(
                    w_sb[:, lo:lo + 4, :],
                    w_d[lo * 128:(lo + 4) * 128, :]
                    .rearrange("(t p) m -> p t m", p=128).bitcast(F32R))

            def dma_w_q(w_sb, w_d, lo):
                nc.sync.dma_start(
                    w_sb[:, lo:lo + 2, :],
                    w_d[lo * 128:(lo + 2) * 128, :]
                    .rearrange("(t p) m -> p t m", p=128).bitcast(F32R))

            for lo in range(0, FT, 2):
                dma_w_q(qtw, qt_d, lo)
                dma_w_q(ktw, kt_d, lo)
                for t in (lo, lo + 1):
                    nc.sync.dma_start(
                        xt_big[:, t, 0:256],
                        xt_d[t * 128:(t + 1) * 128, 0:256].bitcast(F32R))
            for c in range(1, NC_CH):
                cs = slice(c * 256, (c + 1) * 256)
                nc.sync.dma_start(
                    xt_big[:, :, cs],
                    xt_d[:, cs].rearrange("(t p) m -> p t m", p=128)
                    .bitcast(F32R))
            dma_w_half(vtw, vt_d, 0)
            dma_w_half(vtw, vt_d, 4)
            for h in range(HPC):
                nc.sync.dma_start(kt2[h][64:65, :], ones_row[:].bitcast(F32R))

            pts = {}
            negmaxs = {}
            packs_all = {}
            for p in range(2):
                for ty in (0, 1):
                    packs_all[(ty, p)] = pack_pool.tile(
                        [128, S], F32R, tag="pack", name=f"pack{ty}_{p}",
                        bufs=4)

            def emit_proj_chunk(c):
                # all 4 (type, pair) q/k chains for S-cols [c*256,(c+1)*256).
                # pair p covers heads 2p (psum rows 0:64) / 2p+1 (64:128).
                cs = slice(c * 256, (c + 1) * 256)
                chains = []
                for p in (0, 1):
                    for w_sb, ty in ((qtw, 0), (ktw, 1)):
                        ps = pp.tile([128, 256], F32, tag="ps", name="ps")
                        chains.append((ps, w_sb, ty, p))
                for t in range(FT):
                    for ps, w_sb, ty, p in chains:
                        nc.tensor.matmul(
                            ps[:],
                            w_sb[:, t, p * 128:(p + 1) * 128],
                            xt_big[:, t, cs],
                            start=(t == 0), stop=(t == FT - 1),
                        )
                for idx, (ps, w_sb, ty, p) in enumerate(chains):
                    nc.scalar.copy(packs_all[(ty, p)][:, cs],
                                   ps[:].bitcast(F32R))

            def emit_vproj_block(sb_i):
                ps = pp.tile([128, 256], F32, tag="ps", name="vps")
                for t in range(FT):
                    nc.tensor.matmul(
                        ps[:],
                        xt_big[:, t, sb_i * 128:(sb_i + 1) * 128],
                        vtw[:, t, :],
                        start=(t == 0), stop=(t == FT - 1),
                    )
                nc.scalar.copy(
                    v_sb[sb_i][:, :, 0:64],
                    ps[:].rearrange("p (h d) -> p h d", d=64),
                )

            def emit_split(h):
                # per-head 65-row tiles: rows 0:64 from the packed staging
                # tiles (row 64 holds ones / negrow). Split per 512-col
                # piece so each DMA depends on exactly the pack copies that
                # produced its columns.
                p, po = h // 2, 64 * (h % 2)
                for ty, dstl in ((0, qt2), (1, kt2)):
                    pk = packs_all[(ty, p)]
                    nc.sync.dma_start(dstl[h][0:64, :], pk[po:po + 64, :])

            def emit_A_block(h, i):
                if h < 2:
                    pk_q = packs_all[(0, h // 2)]
                    pk_k = packs_all[(1, h // 2)]
                    po = 64 * (h % 2)
                else:
                    pk_q, pk_k, po = qt2[h], kt2[h], 0
                negmax = negmaxs[h]
                w = (i + 1) * 128
                cks = _a_chunks(w)
                nch = len(cks)
                mp = ph2.tile([128, 4], F32, tag="maxpart",
                              name=f"mp{h}", bufs=6)
                tiles = []
                co = 0
                for c, cw in enumerate(cks):
                    sA = psA.tile([128, 512], F32, tag="sA", name=f"sA{h}",
                                  bufs=3)
                    tiles.append((sA, cw))
                    nc.tensor.matmul(
                        sA[:, 0:cw],
                        pk_q[po:po + 64, i * 128:(i + 1) * 128],
                        pk_k[po:po + 64, co:co + cw],
                        start=True, stop=True,
                        tile_position=(po, 0),
                    )
                    if c == nch - 1:  # diag: += -BIG*[k>q] via PE
                        dlo = cw - 128
                        nc.tensor.matmul(
                            sA[:, dlo:dlo + 128],
                            rsh_sb[:], tri_sb[:],
                            start=False, stop=True,
                            skip_group_check=True)
                    co += cw
                if nch == 1:
                    a, acw = tiles[0]
                    nc.vector.reduce_max(
                        negmax[:, i:i + 1], a[:, 0:acw],
                        axis=mybir.AxisListType.X, negate=True)
                else:
                    for c, (a, acw) in enumerate(tiles):
                        nc.vector.reduce_max(
                            mp[:, c:c + 1], a[:, 0:acw],
                            axis=mybir.AxisListType.X)
                    nc.vector.reduce_max(
                        negmax[:, i:i + 1], mp[:, 0:nch],
                        axis=mybir.AxisListType.X, negate=True)

            def emit_negmax_finish(h):
                # negmax -> negrow [1, S] stored as row 64 of qt2[h]
                negmax = negmaxs[h]
                pst = psA.tile([16, 128], F32, tag="sA", bufs=3)
                nc.tensor.transpose(pst[:], negmax[:], identf[:])
                stage = ph2.tile([16, 128], F32, tag="stage", bufs=2)
                nc.vector.tensor_copy(stage[:], pst[:])
                nc.sync.dma_start(qt2[h][64:65, :], stage[:].bitcast(F32R))

            def A_thunks(h):
                negmaxs[h] = ph2.tile([128, NB], F32, tag="negmax",
                                      name=f"negmax{h}", bufs=4)
                out = [(lambda hh=h, ii=i: emit_A_block(hh, ii))
                       for i in range(NB)]
                out.append(lambda hh=h: emit_negmax_finish(hh))
                return out

            def T_thunks(h):
                pt = pt_pool.tile([128, PT_COLS], BF16, tag="pt",
                                  name=f"pt{h}")
                pts[h] = pt

                def tile_thunk(j, t0, tw):
                    def f():
                        sT = psT.tile([128, 1024], F32, tag="sT",
                                      name=f"sT{h}")
                        coff = 0
                        # bank-aligned 512 splits: a matmul's psum output
                        # must not cross a 2KB bank boundary
                        tcks = [512] * (tw // 512)
                        if tw % 512:
                            tcks.append(tw % 512)
                        for cw in tcks:
                            diag = (t0 == j * 128 and coff == 0)
                            # K=65: row 64 = ones (kt2) x negrow (qt2)
                            # -> scores - rowmax, fused
                            nc.tensor.matmul(
                                sT[:, coff:coff + cw],
                                kt2[h][0:65, j * 128:(j + 1) * 128],
                                qt2[h][0:65, t0 + coff:t0 + coff + cw],
                                start=True, stop=not diag,
                            )
                            if diag:  # += -BIG*[q<k] via PE
                                nc.tensor.matmul(
                                    sT[:, 0:128],
                                    tri_sb[:], rsh_sb[:],
                                    start=False, stop=True,
                                    skip_group_check=True)
                            coff += cw
                        nc.scalar.activation(
                            pt[:, PT_OFF[j] + t0 - j * 128:
                               PT_OFF[j] + t0 - j * 128 + tw],
                            sT[:, 0:tw],
                            mybir.ActivationFunctionType.Exp)
                    return f

                out = []   # list of (j_completed_after, thunk)
                for j in range(NB):
                    t0 = j * 128
                    while t0 < S:
                        tw = min(1024, S - t0)
                        last = (t0 + tw == S)
                        out.append((j if last else j - 1,
                                    tile_thunk(j, t0, tw)))
                        t0 += tw
                return out

            def emit_av_block(h, i, with_ph3):
                pt = pts[h]
                av = av2[:, i % 3, :]
                for j in range(i + 1):
                    nc.tensor.matmul(
                        av[:],
                        pt[:, PT_OFF[j] + (i - j) * 128:
                           PT_OFF[j] + (i - j) * 128 + 128],
                        v_sb[j][:, h, :],
                        start=(j == 0), stop=(j == i),
                    )
                recip = ph2.tile([128, 1], F32, tag="recip", bufs=6)
                nc.vector.reciprocal(recip[:], av[:, 64:65])
                nc.vector.tensor_scalar_mul(
                    ho_sb[i][:, h * 64:(h + 1) * 64],
                    av[:, 0:64], recip[:])
                if with_ph3:
                    emit_ph3(i)

            def av_thunks(h, with_ph3=False):
                return [(lambda hh=h, ii=i, w3=with_ph3:
                         emit_av_block(hh, ii, w3)) for i in range(NB)]

            def emit_ph3(i):
                hot = ph3.tile([128, 256], BF16, tag="hot", name="hot",
                               bufs=2)
                for t in range(2):
                    ptile = psA.tile([128, 128], BF16, tag="sA",
                                     name="ptile", bufs=3)
                    nc.tensor.transpose(
                        ptile[:], ho_sb[i][:, t * 128:(t + 1) * 128],
                        identb[:])
                    nc.vector.tensor_copy(hot[:, t * 128:(t + 1) * 128],
                                          ptile[:])
                ostage = ph3.tile([128, D], F32, tag="ostage", name="ostage",
                                  bufs=2)
                for nchunk in range(2):
                    pot = psA.tile([128, 512], F32, tag="sA", name="pot",
                                   bufs=3)
                    for t in range(2):
                        nc.tensor.matmul(
                            pot[:],
                            hot[:, t * 128:(t + 1) * 128],
                            ot_sb[:, t, nchunk * 512:(nchunk + 1) * 512],
                            start=(t == 0), stop=(t == 1),
                        )
                    ocs = slice(nchunk * 512, (nchunk + 1) * 512)
                    if nchunk == 0:
                        nc.scalar.copy(ostage[:, ocs], pot[:])
                    else:
                        nc.vector.tensor_copy(ostage[:, ocs], pot[:])
                    nc.sync.dma_start(out_d[i * 128:(i + 1) * 128, ocs],
                                      ostage[:, ocs])

            # ---------------- phase 1: cadenced with the x stream --------
            # per chunk c: ready v-proj blocks first (front-loads deferrable
            # work), then q/k chains (gated by chunk c's DMA), then the
            # A(0)/A(1) blocks whose k-range chunk c completes. This keeps
            # the post-stream PE backlog minimal so T(0)'s exp can start as
            # soon as negrow(0) lands.
            a01 = {0: A_thunks(0), 1: A_thunks(1)}
            for c in range(NC_CH):
                emit_proj_chunk(c)
                if c < NC_CH - 1:
                    for h in (0, 1):
                        a01[h][2 * c]()
                        a01[h][2 * c + 1]()
            for h in (0, 1):
                a01[h][2 * (NC_CH - 1)]()
                a01[h][2 * (NC_CH - 1) + 1]()
            # splits first: they depend only on the packs (ready early) and
            # must not queue behind the negrow DMAs on the SP queue
            for h in range(HPC):
                emit_split(h)
            for h in (0, 1):
                a01[h][NB]()               # negmax finish

            # v-proj last: it gates only av(0), so it overlaps the
            # negrow/split latency and the start of T(0)'s exp stream.
            for sb_i in range(NB):
                emit_vproj_block(sb_i)
            pp_cm.__exit__(None, None, None)    # frees 4 PSUM banks
            ph1_cm.__exit__(None, None, None)   # frees x/weights SBUF
            pack_cm.__exit__(None, None, None)  # frees 32KB of staging

            pt_cm = tc.tile_pool(name="pt_pool", bufs=2)
            ph3_cm = tc.tile_pool(name="ph3", bufs=4)
            psT_cm = tc.tile_pool(name="psT", bufs=2, space="PSUM")
            psV_cm = tc.tile_pool(name="psV", bufs=1, space="PSUM")
            pt_pool, ph3 = pt_cm.__enter__(), ph3_cm.__enter__()
            psT, psV = psT_cm.__enter__(), psV_cm.__enter__()
            av2 = psV.tile([128, 3, 65], F32, tag="av", name="av2", bufs=1)
            ot_sb = ph3.tile([128, 2, D], BF16, tag="ot_sb", bufs=1)
            nc.gpsimd.dma_start(ot_sb[:],
                                ot_d[:].rearrange("(t p) n -> p t n", p=128))

            # ---------------- phase 2: T windows with fillers ------------
            # T(0) window: A(2) blocks fill the exp-paced bubbles
            _interleave([t for _, t in T_thunks(0)], A_thunks(2))
            # T(1) window: av(0) zipped with A(3) so A(3)'s DVE reduces
            # start early enough that negrow(3) doesn't gate T(3)
            av0t, a3t = av_thunks(0), A_thunks(3)
            zipped = [t for pair in zip(av0t, a3t) for t in pair] + a3t[16:]
            _interleave([t for _, t in T_thunks(1)], zipped)
            # T(2) window: av(1)
            _interleave([t for _, t in T_thunks(2)], av_thunks(1))
            # T(3) window: av(2), plus av(3)+ph3 blocks as soon as the PT
            # columns they need are exp'd (j-coverage pacing)
            t3 = T_thunks(3)
            av2_t = av_thunks(2)
            av3_t = av_thunks(3)
            fi2 = 0
            nxt3 = 0
            for k, (jdone, thunk) in enumerate(t3):
                thunk()
                want = (k + 1) * len(av2_t) // len(t3)
                while fi2 < want:
                    av2_t[fi2]()
                    fi2 += 1
                while nxt3 <= jdone:
                    av3_t[nxt3]()
                    if nxt3 >= 1:
                        emit_ph3(nxt3 - 1)
                    nxt3 += 1
            while fi2 < len(av2_t):
                av2_t[fi2]()
                fi2 += 1
            while nxt3 < NB:
                av3_t[nxt3]()
                if nxt3 >= 1:
                    emit_ph3(nxt3 - 1)
                nxt3 += 1
            emit_ph3(NB - 1)

            for cm in (psV_cm, psT_cm, ph3_cm, pt_cm, psA_cm, ph2_cm):
                cm.__exit__(None, None, None)

    nc.compile()
    return nc


_NC_CACHE = None


def _get_nc():
    global _NC_CACHE
    if _NC_CACHE is None:
        _NC_CACHE = build_nc()
    return _NC_CACHE


def kernel(x, Q, K, V, O, num_heads=16, _want_results=False, **run_kwargs):
    x = np.asarray(x, dtype=np.float32)
    Q = np.asarray(Q, dtype=np.float32)
    K = np.asarray(K, dtype=np.float32)
    V = np.asarray(V, dtype=np.float32)
    O = np.asarray(O, dtype=np.float32)
    assert x.shape == (B, S, D) and int(num_heads) == H

    idx = np.arange(128)
    # tri[c,k] = [c<=k]; rsh[c,q] = -BIG*[c==q+1]
    # A-side: (rsh.T@tri)[q,k] = -BIG*[k>q]; T-side: (tri.T@rsh)[k,q] = -BIG*[q<k]
    tri = (idx[:, None] <= idx[None, :]).astype(np.float32)
    rsh = np.zeros((128, 128), dtype=np.float32)
    rsh[idx[1:], idx[:-1]] = NEG

    in_maps = []
    for c in range(8):
        b, g = c // 4, c % 4
        rows = slice(g * 256, (g + 1) * 256)
        in_maps.append(dict(
            xt=np.ascontiguousarray(x[b].T),
            qt=np.ascontiguousarray((Q[rows, :] / 8.0).T),
            kt=np.ascontiguousarray(K[rows, :].T),
            vt=np.ascontiguousarray(V[rows, :].T),
            ot=np.ascontiguousarray(O[:, rows].T),
            tri=tri,
            rsh=rsh,
        ))

    nc = _get_nc()
    res = run_bass_kernel_spmd(nc, in_maps, core_ids=list(range(8)), **run_kwargs)

    out = np.zeros((B, S, D), dtype=np.float32)
    for c in range(8):
        out[c // 4] += res.results[c]["out"]
    if _want_results:
        return out, res
    return out
